# revision 13
# baseline (speedup 1.0000x reference)
"""Trainium2 Bass kernel for the batched 2D Kalman filter (nn_KalmanFilterWrapper).

Math
----
The reference runs, per trajectory, a Kalman filter over T=4096 steps with a
constant-velocity model.  The gain/covariance recursion (Riccati) is
data-independent, so the scan collapses to a linear time-varying recurrence
with coefficients shared across the whole batch; the 4-state filter decouples
into two identical 2-state scalar filters (one per coordinate), giving
B*2 = 8192 independent scalar sequences.

Blocking time into chunks of C=126 steps turns the filter into one
[128x128] @ [128x512] matmul per block and PSUM chunk: contract rows 0,1
carry the filter state from the previous block, rows 2+j the block's
measurements; output rows 0,1 duplicate the end-of-block state (next block's
carry), rows 2+j the filtered positions.  The Riccati recursion reaches
steady state inside block 0, so only two coefficient matrices exist: block 0
(init folded in, carry coefficients zero — the host zero-fills those rows)
and the shared steady-state matrix used by every other block, including the
short zero-padded last one.  Coefficients are precomputed on the host in
float64.

Layout / engines
----------------
Everything on the wire is float16: the rel-err budget (2e-2) dwarfs the
~3.6e-4 this costs, it halves HBM traffic (the kernel is DMA-bound), and
fp16 matmuls run the PE at full rate.

DMA descriptor generation is serial on the issuing sequencer (~7 ns/desc),
so both z and v live in DRAM as [128, NBLK*NCOLS] SLABS: partition row
2+j holds step j of every block side by side.  One DMA then moves a GROUP
of up to 4 consecutive blocks with just 128 descriptors of contiguous
8 KB lines (vs 126 x 2 KB per block in [time, batch] layout), cutting
trigger-side generation ~4x.  Group sizes ramp 1,1,2,4,... so the first
matmul isn't gated on a full group load.  The host packs/unpacks the slabs.

The serial carry chain stays short: ACT copies the matmul's state rows into
the next block's carry slot ([2 x 512] per chunk) while ALL full-tile
PSUM->SBUF evictions run on DVE, so the chain never queues behind an
eviction.

Sharding: data-parallel across 8 NeuronCores, 512 trajectories (1024 scalar
sequences) per core.
"""

import numpy as np

import concourse.bass as bass
import concourse.bacc as bacc
import concourse.mybir as mybir
from concourse.bass_utils import run_bass_kernel_spmd
from concourse.tile import TileContext

# Problem constants (hardcoded per harness contract).
B = 4096
T = 4096
DT = 1.0
PROCESS_VARIANCE = 1e-05
MEASUREMENT_VARIANCE = 0.1
INIT_ERROR = 1.0

N_CORES = 8
NCOLS = (B * 2) // N_CORES  # 1024 scalar sequences per core
MAIN_C = 126                # steps per block
LAST_C = T - (T // MAIN_C) * MAIN_C  # 64 (block 32, zero-padded to 126)
NBLK = T // MAIN_C + (1 if LAST_C else 0)  # 33
CHUNK = 512                 # matmul moving free-dim (one fp32 PSUM bank)

# blocks per DMA group: ramp in for a fast first matmul, 1-block tail
GSIZES = [1, 1, 2] + [4] * 7 + [1]
assert sum(GSIZES) == NBLK
GBASE = np.cumsum([0] + GSIZES).tolist()
GMAX = max(GSIZES)

DT_F16 = mybir.dt.float16
DT_F32 = mybir.dt.float32


def _riccati():
    F = np.array([[1.0, DT], [0.0, 1.0]], dtype=np.float64)
    I2 = np.eye(2, dtype=np.float64)
    P = INIT_ERROR * I2.copy()
    A = np.zeros((T, 2, 2), dtype=np.float64)
    k = np.zeros((T, 2), dtype=np.float64)
    for t in range(T):
        Pp = F @ P @ F.T + PROCESS_VARIANCE * I2
        s = Pp[0, 0] + MEASUREMENT_VARIANCE
        kt = Pp[:, 0] / s
        k[t] = kt
        KH = np.zeros((2, 2), dtype=np.float64)
        KH[:, 0] = kt
        P = (I2 - KH) @ Pp
        A[t] = (I2 - KH) @ F
    return A, k


def _precompute_u():
    """Returns U [128, 256] f16 in lhsT layout (U[i, m] = coefficient of
    contract input i in output m).  Cols 0:128 = block 0 (init folded, carry
    rows zero), cols 128:256 = steady-state block.  Contract rows 0,1 =
    carry, 2+j = z_j; out cols 0 = p_last, 1 = v_last, 2+j = p_j."""
    A, k = _riccati()
    U = np.zeros((128, 256), dtype=np.float64)
    for sl, t0 in ((0, 0), (1, MAIN_C)):
        L = U[:, sl * 128:(sl + 1) * 128]
        Rc = np.zeros((2, 128), dtype=np.float64)
        if sl == 0:
            Rc[0, 2] = 1.0  # x_{-1} = [z_0, 0]; z_0 is contract input 2
        else:
            Rc[0, 0] = 1.0  # carry row 0 = p_prev
            Rc[1, 1] = 1.0  # carry row 1 = v_prev
        for j in range(MAIN_C):
            t = t0 + j
            Rc = A[t] @ Rc
            Rc[:, 2 + j] += k[t]
            L[:, 2 + j] = Rc[0, :]
        L[:, 0] = Rc[0, :]  # p_last (dup) -> next block carry row 0
        L[:, 1] = Rc[1, :]  # v_last      -> next block carry row 1
    return np.ascontiguousarray(U.astype(np.float16))


def _build_nc():
    nchunks = NCOLS // CHUNK
    ngroups = len(GSIZES)

    nc = bacc.Bacc()
    # group 0's measurements and the coefficients ride one DMA: z0u cols
    # 0:NCOLS = block 0 (carry rows host-zeroed), NCOLS:NCOLS+256 = U
    z0u = nc.dram_tensor("z0u", [128, NCOLS + 256], DT_F16, kind="ExternalInput")
    z = nc.dram_tensor("z", [128, NBLK * NCOLS], DT_F16, kind="ExternalInput")
    v = nc.dram_tensor("v", [128, NBLK * NCOLS], DT_F16, kind="ExternalOutput")

    with TileContext(nc) as tc:
        with (
            tc.tile_pool(name="consts", bufs=1) as cpool,
            tc.tile_pool(name="zpool", bufs=4) as zpool,
            tc.tile_pool(name="vpool", bufs=4) as vpool,
            tc.tile_pool(name="psum", bufs=8, space="PSUM") as ppool,
        ):
            gtiles = {}

            # SWDGE (gpsimd) rides the two DMA engines the HWDGE rings never
            # touch; that pool moves ~45 GB/s, so it gets whole early groups
            # and single blocks of late ones, sized so nothing arrives late.
            SWDGE_BLOCKS = {5: 4, 7: 4}

            def fetch_group(g):
                # carry rows 0,1 are ACT-written, not loaded (group 0's came
                # from the host inside z0u)
                gs = GSIZES[g]
                sw = min(SWDGE_BLOCKS.get(g, 0), gs)
                gt = zpool.tile([128, GMAX * NCOLS], DT_F16, tag="zg")
                hw = gs - sw
                if hw:
                    nc.sync.dma_start(
                        gt[2:128, 0:hw * NCOLS],
                        z[2:128, GBASE[g] * NCOLS:(GBASE[g] + hw) * NCOLS],
                    )
                if sw:
                    nc.gpsimd.dma_start(
                        gt[2:128, hw * NCOLS:gs * NCOLS],
                        z[2:128, (GBASE[g] + hw) * NCOLS:(GBASE[g] + gs) * NCOLS],
                    )
                gtiles[g] = gt

            def ztile_of(b):
                """(tile, col offset) holding block b."""
                g = 0
                while GBASE[g + 1] <= b:
                    g += 1
                return gtiles[g], (b - GBASE[g]) * NCOLS

            g0u = cpool.tile([128, NCOLS + 256], DT_F16)
            nc.sync.dma_start(g0u[:, :], z0u[:, :])
            gtiles[0] = g0u
            u_t = g0u  # coefficient cols live at NCOLS + [0, 256)
            for g in range(1, 5):
                fetch_group(g)

            for g in range(ngroups):
                gs = GSIZES[g]
                vgt = vpool.tile([128, GMAX * NCOLS], DT_F16, tag="vg")
                gout0 = 0  # first block of the group not yet DMA'd out
                for q in range(gs):
                    b = GBASE[g] + q
                    zgt = gtiles[g]
                    usel = bass.ds(NCOLS, 128) if b == 0 else bass.ds(NCOLS + 128, 128)
                    pss = []
                    for ci in range(nchunks):
                        zcols = bass.ds(q * NCOLS + ci * CHUNK, CHUNK)
                        ps = ppool.tile([128, CHUNK], DT_F32)
                        nc.tensor.matmul(
                            ps[:, :], u_t[0:128, usel], zgt[0:128, zcols],
                            start=True, stop=True,
                        )
                        # state rows -> next block's carry slot, on ACT (off
                        # the eviction path so the serial chain stays short)
                        if b + 1 < NBLK:
                            nt, noff = ztile_of(b + 1)
                            nc.scalar.copy(
                                nt[0:2, bass.ds(noff + ci * CHUNK, CHUNK)],
                                ps[0:2, :],
                            )
                        pss.append((ps, zcols))
                    # full-tile evictions all on DVE
                    for ps, cols in pss:
                        nc.vector.tensor_copy(vgt[:, cols], ps[:, :])
                    # drain finished pairs of blocks early so outputs never
                    # bunch up at the end of the run
                    if q == gs - 1 or q == gout0 + 1:
                        c0, c1 = GBASE[g] + gout0, GBASE[g] + q + 1
                        nc.sync.dma_start(
                            v[2:128, c0 * NCOLS:c1 * NCOLS],
                            vgt[2:128, gout0 * NCOLS:(q + 1) * NCOLS],
                        )
                        gout0 = q + 1
                if g + 5 < ngroups:
                    fetch_group(g + 5)
    nc.finalize()
    return nc


_CACHE = {}


def _pack_z(x):
    """[B, T, 2] f32 -> slab [128, NBLK, B*2] f16:
    row 2+j, slab b = measurements at step b*126+j (zero-padded)."""
    zt = x.transpose(1, 0, 2).reshape(T, B * 2).astype(np.float16)
    ztp = np.zeros((NBLK * MAIN_C, B * 2), np.float16)
    ztp[:T] = zt
    slab = np.zeros((128, NBLK, B * 2), np.float16)
    slab[2:128] = ztp.reshape(NBLK, MAIN_C, B * 2).transpose(1, 0, 2)
    return slab


def _unpack_v(v_slab):
    """slab [128, NBLK, B*2] f16 -> [B, T, 2] f32."""
    vt = v_slab[2:128].transpose(1, 0, 2).reshape(NBLK * MAIN_C, B * 2)[:T]
    return np.ascontiguousarray(
        vt.astype(np.float32).reshape(T, B, 2).transpose(1, 0, 2))


def _run(x_seq: np.ndarray, trace: bool = False):
    if "nc" not in _CACHE:
        _CACHE["nc"] = _build_nc()
        _CACHE["u"] = _precompute_u()
    nc = _CACHE["nc"]
    u_all = _CACHE["u"]

    x = np.asarray(x_seq)
    assert x.shape == (B, T, 2), x.shape

    slab = _pack_z(x)
    in_maps = []
    for i in range(N_CORES):
        zi = np.ascontiguousarray(
            slab[:, :, i * NCOLS:(i + 1) * NCOLS]).reshape(128, NBLK * NCOLS)
        z0u = np.concatenate([zi[:, 0:NCOLS], u_all], axis=1)
        in_maps.append({"z": zi, "z0u": np.ascontiguousarray(z0u)})
    res = run_bass_kernel_spmd(nc, in_maps, core_ids=list(range(N_CORES)), trace=trace)

    v_slab = np.concatenate(
        [r["v"].reshape(128, NBLK, NCOLS) for r in res.results], axis=2)
    return _unpack_v(v_slab), res


def kernel(x_seq: np.ndarray) -> np.ndarray:
    out, _ = _run(x_seq, trace=False)
    return out


# revision 14
# speedup vs baseline: 1.0086x; 1.0086x over previous
"""Trainium2 Bass kernel for the batched 2D Kalman filter (nn_KalmanFilterWrapper).

Math
----
The reference runs, per trajectory, a Kalman filter over T=4096 steps with a
constant-velocity model.  The gain/covariance recursion (Riccati) is
data-independent, so the scan collapses to a linear time-varying recurrence
with coefficients shared across the whole batch; the 4-state filter decouples
into two identical 2-state scalar filters (one per coordinate), giving
B*2 = 8192 independent scalar sequences.

Blocking time into chunks of C=126 steps turns the filter into one
[128x128] @ [128x512] matmul per block and PSUM chunk: contract rows 0,1
carry the filter state from the previous block, rows 2+j the block's
measurements; output rows 0,1 duplicate the end-of-block state (next block's
carry), rows 2+j the filtered positions.  The Riccati recursion reaches
steady state inside block 0, so only two coefficient matrices exist: block 0
(init folded in, carry coefficients zero — the host zero-fills those rows)
and the shared steady-state matrix used by every other block, including the
short zero-padded last one.  Coefficients are precomputed on the host in
float64.

Layout / engines
----------------
Everything on the wire is float16: the rel-err budget (2e-2) dwarfs the
~3.6e-4 this costs, it halves HBM traffic (the kernel is DMA-bound), and
fp16 matmuls run the PE at full rate.

DMA descriptor generation is serial on the issuing sequencer (~7 ns/desc),
so both z and v live in DRAM as [128, NBLK*NCOLS] SLABS: partition row
2+j holds step j of every block side by side.  One DMA then moves a GROUP
of up to 4 consecutive blocks with just 128 descriptors of contiguous
8 KB lines (vs 126 x 2 KB per block in [time, batch] layout), cutting
trigger-side generation ~4x.  Group sizes ramp 1,1,2,4,... so the first
matmul isn't gated on a full group load.  The host packs/unpacks the slabs.

The serial carry chain stays short: ACT copies the matmul's state rows into
the next block's carry slot ([2 x 512] per chunk) while ALL full-tile
PSUM->SBUF evictions run on DVE, so the chain never queues behind an
eviction.

Sharding: data-parallel across 8 NeuronCores, 512 trajectories (1024 scalar
sequences) per core.
"""

import numpy as np

import concourse.bass as bass
import concourse.bacc as bacc
import concourse.mybir as mybir
from concourse.bass_utils import run_bass_kernel_spmd
from concourse.tile import TileContext

# Problem constants (hardcoded per harness contract).
B = 4096
T = 4096
DT = 1.0
PROCESS_VARIANCE = 1e-05
MEASUREMENT_VARIANCE = 0.1
INIT_ERROR = 1.0

N_CORES = 8
NCOLS = (B * 2) // N_CORES  # 1024 scalar sequences per core
MAIN_C = 126                # steps per block
LAST_C = T - (T // MAIN_C) * MAIN_C  # 64 (block 32, zero-padded to 126)
NBLK = T // MAIN_C + (1 if LAST_C else 0)  # 33
CHUNK = 512                 # matmul moving free-dim (one fp32 PSUM bank)

# blocks per DMA group: ramp in for a fast first matmul, 1-block tail
GSIZES = [1, 1, 3] + [4] * 7
assert sum(GSIZES) == NBLK
GBASE = np.cumsum([0] + GSIZES).tolist()
GMAX = max(GSIZES)

DT_F16 = mybir.dt.float16
DT_F32 = mybir.dt.float32


def _riccati():
    F = np.array([[1.0, DT], [0.0, 1.0]], dtype=np.float64)
    I2 = np.eye(2, dtype=np.float64)
    P = INIT_ERROR * I2.copy()
    A = np.zeros((T, 2, 2), dtype=np.float64)
    k = np.zeros((T, 2), dtype=np.float64)
    for t in range(T):
        Pp = F @ P @ F.T + PROCESS_VARIANCE * I2
        s = Pp[0, 0] + MEASUREMENT_VARIANCE
        kt = Pp[:, 0] / s
        k[t] = kt
        KH = np.zeros((2, 2), dtype=np.float64)
        KH[:, 0] = kt
        P = (I2 - KH) @ Pp
        A[t] = (I2 - KH) @ F
    return A, k


def _precompute_u():
    """Returns U [128, 256] f16 in lhsT layout (U[i, m] = coefficient of
    contract input i in output m).  Cols 0:128 = block 0 (init folded, carry
    rows zero), cols 128:256 = steady-state block.  Contract rows 0,1 =
    carry, 2+j = z_j; out cols 0 = p_last, 1 = v_last, 2+j = p_j."""
    A, k = _riccati()
    U = np.zeros((128, 256), dtype=np.float64)
    for sl, t0 in ((0, 0), (1, MAIN_C)):
        L = U[:, sl * 128:(sl + 1) * 128]
        Rc = np.zeros((2, 128), dtype=np.float64)
        if sl == 0:
            Rc[0, 2] = 1.0  # x_{-1} = [z_0, 0]; z_0 is contract input 2
        else:
            Rc[0, 0] = 1.0  # carry row 0 = p_prev
            Rc[1, 1] = 1.0  # carry row 1 = v_prev
        for j in range(MAIN_C):
            t = t0 + j
            Rc = A[t] @ Rc
            Rc[:, 2 + j] += k[t]
            L[:, 2 + j] = Rc[0, :]
        L[:, 0] = Rc[0, :]  # p_last (dup) -> next block carry row 0
        L[:, 1] = Rc[1, :]  # v_last      -> next block carry row 1
    return np.ascontiguousarray(U.astype(np.float16))


def _build_nc():
    nchunks = NCOLS // CHUNK
    ngroups = len(GSIZES)

    nc = bacc.Bacc()
    # group 0's measurements and the coefficients ride one DMA: z0u cols
    # 0:NCOLS = block 0 (carry rows host-zeroed), NCOLS:NCOLS+256 = U
    z0u = nc.dram_tensor("z0u", [128, NCOLS + 256], DT_F16, kind="ExternalInput")
    z = nc.dram_tensor("z", [128, NBLK * NCOLS], DT_F16, kind="ExternalInput")
    v = nc.dram_tensor("v", [128, NBLK * NCOLS], DT_F16, kind="ExternalOutput")

    with TileContext(nc) as tc:
        with (
            tc.tile_pool(name="consts", bufs=1) as cpool,
            tc.tile_pool(name="zpool", bufs=4) as zpool,
            tc.tile_pool(name="vpool", bufs=4) as vpool,
            tc.tile_pool(name="psum", bufs=8, space="PSUM") as ppool,
        ):
            gtiles = {}

            # SWDGE (gpsimd) rides the two DMA engines the HWDGE rings never
            # touch; that pool moves ~45 GB/s, so it gets whole early groups
            # and single blocks of late ones, sized so nothing arrives late.
            SWDGE_BLOCKS = {6: 4, 8: 4}

            def fetch_group(g):
                # carry rows 0,1 are ACT-written, not loaded (group 0's came
                # from the host inside z0u)
                gs = GSIZES[g]
                sw = min(SWDGE_BLOCKS.get(g, 0), gs)
                gt = zpool.tile([128, GMAX * NCOLS], DT_F16, tag="zg")
                hw = gs - sw
                if hw:
                    nc.sync.dma_start(
                        gt[2:128, 0:hw * NCOLS],
                        z[2:128, GBASE[g] * NCOLS:(GBASE[g] + hw) * NCOLS],
                    )
                if sw:
                    nc.gpsimd.dma_start(
                        gt[2:128, hw * NCOLS:gs * NCOLS],
                        z[2:128, (GBASE[g] + hw) * NCOLS:(GBASE[g] + gs) * NCOLS],
                    )
                gtiles[g] = gt

            def ztile_of(b):
                """(tile, col offset) holding block b."""
                g = 0
                while GBASE[g + 1] <= b:
                    g += 1
                return gtiles[g], (b - GBASE[g]) * NCOLS

            g0u = cpool.tile([128, NCOLS + 256], DT_F16)
            nc.sync.dma_start(g0u[:, :], z0u[:, :])
            gtiles[0] = g0u
            u_t = g0u  # coefficient cols live at NCOLS + [0, 256)
            for g in range(1, 5):
                fetch_group(g)

            for g in range(ngroups):
                gs = GSIZES[g]
                vgt = vpool.tile([128, GMAX * NCOLS], DT_F16, tag="vg")
                gout0 = 0  # first block of the group not yet DMA'd out
                for q in range(gs):
                    b = GBASE[g] + q
                    zgt = gtiles[g]
                    usel = bass.ds(NCOLS, 128) if b == 0 else bass.ds(NCOLS + 128, 128)
                    pss = []
                    for ci in range(nchunks):
                        zcols = bass.ds(q * NCOLS + ci * CHUNK, CHUNK)
                        ps = ppool.tile([128, CHUNK], DT_F32)
                        nc.tensor.matmul(
                            ps[:, :], u_t[0:128, usel], zgt[0:128, zcols],
                            start=True, stop=True,
                        )
                        # state rows -> next block's carry slot, on ACT (off
                        # the eviction path so the serial chain stays short)
                        if b + 1 < NBLK:
                            nt, noff = ztile_of(b + 1)
                            nc.scalar.copy(
                                nt[0:2, bass.ds(noff + ci * CHUNK, CHUNK)],
                                ps[0:2, :],
                            )
                        pss.append((ps, zcols))
                    # full-tile evictions all on DVE
                    for ps, cols in pss:
                        nc.vector.tensor_copy(vgt[:, cols], ps[:, :])
                    # drain finished pairs of blocks early so outputs never
                    # bunch up at the end of the run
                    if q == gs - 1 or q == gout0 + 1:
                        c0, c1 = GBASE[g] + gout0, GBASE[g] + q + 1
                        nc.sync.dma_start(
                            v[2:128, c0 * NCOLS:c1 * NCOLS],
                            vgt[2:128, gout0 * NCOLS:(q + 1) * NCOLS],
                        )
                        gout0 = q + 1
                if g + 5 < ngroups:
                    fetch_group(g + 5)
    nc.finalize()
    return nc


_CACHE = {}


def _pack_z(x):
    """[B, T, 2] f32 -> slab [128, NBLK, B*2] f16:
    row 2+j, slab b = measurements at step b*126+j (zero-padded)."""
    zt = x.transpose(1, 0, 2).reshape(T, B * 2).astype(np.float16)
    ztp = np.zeros((NBLK * MAIN_C, B * 2), np.float16)
    ztp[:T] = zt
    slab = np.zeros((128, NBLK, B * 2), np.float16)
    slab[2:128] = ztp.reshape(NBLK, MAIN_C, B * 2).transpose(1, 0, 2)
    return slab


def _unpack_v(v_slab):
    """slab [128, NBLK, B*2] f16 -> [B, T, 2] f32."""
    vt = v_slab[2:128].transpose(1, 0, 2).reshape(NBLK * MAIN_C, B * 2)[:T]
    return np.ascontiguousarray(
        vt.astype(np.float32).reshape(T, B, 2).transpose(1, 0, 2))


def _run(x_seq: np.ndarray, trace: bool = False):
    if "nc" not in _CACHE:
        _CACHE["nc"] = _build_nc()
        _CACHE["u"] = _precompute_u()
    nc = _CACHE["nc"]
    u_all = _CACHE["u"]

    x = np.asarray(x_seq)
    assert x.shape == (B, T, 2), x.shape

    slab = _pack_z(x)
    in_maps = []
    for i in range(N_CORES):
        zi = np.ascontiguousarray(
            slab[:, :, i * NCOLS:(i + 1) * NCOLS]).reshape(128, NBLK * NCOLS)
        z0u = np.concatenate([zi[:, 0:NCOLS], u_all], axis=1)
        in_maps.append({"z": zi, "z0u": np.ascontiguousarray(z0u)})
    res = run_bass_kernel_spmd(nc, in_maps, core_ids=list(range(N_CORES)), trace=trace)

    v_slab = np.concatenate(
        [r["v"].reshape(128, NBLK, NCOLS) for r in res.results], axis=2)
    return _unpack_v(v_slab), res


def kernel(x_seq: np.ndarray) -> np.ndarray:
    out, _ = _run(x_seq, trace=False)
    return out
